# revision 1
# baseline (speedup 1.0000x reference)
"""Trainium2 Bass kernel for nn_CoAttentionFusionBlock.

Math: the reference's softmax is over a singleton dim, so its weights are
exactly 1.0 and o1/o2 equal the raw features bit-for-bit. The module reduces to

    out = concat([feat_depth, feat_rgb], axis=1) @ W_f.T + b_f        # [B, D]

W_k1/b_k1/W_k2/b_k2 only feed the (dead) score path and are never needed.

Distribution: pure data parallel over the batch dim across 8 NeuronCores.
Each core computes yT = WfT.T @ xT (all operands pre-transposed on host so
the contraction dim lands on SBUF partitions), where
    xT  = concat([feat_depth, feat_rgb], 1).T shard   [2048, 4096]
    WfT = W_f.T                                       [2048, 1024]
    yT  = out shard transposed                        [1024, 4096]

Matmul inputs are declared float32r (fp32 bits, single-pass TF32-style PE
matmul at 1 cycle/row for moving dim >= 256) so the PE runs at full rate
instead of the 4x-slower exact-fp32 hi/lo decomposition. Measured ~256us/core
HW, absmax rel err ~1.6e-4 vs the fp32 reference.
"""

import numpy as np

import concourse.bacc as bacc
import concourse.mybir as mybir
import concourse.tile as tile
from concourse.bass_utils import run_bass_kernel_spmd

B = 32768
D = 1024
NCORES = 8
BLOC = B // NCORES  # 4096 rows per core
K = 2 * D  # 2048 contraction dim
P = 128  # partitions
NT = 512  # moving free dim per matmul (one PSUM bank of fp32)
KT = K // P  # 16 k-tiles
JT = D // P  # 8 output-row tiles
BT = BLOC // NT  # 8 batch tiles

FP32 = mybir.dt.float32
FP32R = mybir.dt.float32r

# test.py can flip these to profile; harness leaves them alone.
TRACE = False
TRACE_DIR = None
LAST_RESULT = None

# Matmul input dtype: "fp32r" (exact fp32 bits, TF32-ish multiply) or "bf16"
# (half the DMA traffic, 2 cols/cycle on the PE) or "mixed" (fp32r weights,
# bf16 activations).
DT_IN = "fp32r"


def _dtypes():
    if DT_IN == "fp32r":
        return FP32R, FP32R, np.float32, np.float32
    import ml_dtypes

    bf16 = np.dtype(ml_dtypes.bfloat16)
    if DT_IN == "bf16":
        return mybir.dt.bfloat16, mybir.dt.bfloat16, bf16, bf16
    if DT_IN == "mixed":
        return FP32R, mybir.dt.bfloat16, np.float32, bf16
    raise ValueError(DT_IN)


def _build_nc():
    # Bacc (not raw Bass): its compile() runs move_matmul_waits_to_ldweights +
    # generate_event_semaphores, which split sync waits to <=1 per instruction
    # (TRN2 HW limit — raw Bass hits "Too many sync wait commands" in walrus).
    nc = bacc.Bacc(None)
    w_dt, x_dt, _, _ = _dtypes()
    xT = nc.declare_dram_parameter("xT", [K, BLOC], x_dt, isOutput=False)
    wT = nc.declare_dram_parameter("wT", [K, D], w_dt, isOutput=False)
    biasT = nc.declare_dram_parameter("biasT", [P, JT], FP32, isOutput=False)
    yT = nc.declare_dram_parameter("yT", [D, BLOC], FP32, isOutput=True)

    # DRAM views with the 128-partition tile dim explicit
    xT_v = xT.rearrange("(t p) b -> p t b", p=P)  # [128, KT, BLOC]
    wT_v = wT.rearrange("(t p) j -> p t j", p=P)  # [128, KT, D]
    yT_v = yT.rearrange("(j p) b -> j p b", p=P)  # [JT, 128, BLOC]

    with tile.TileContext(nc) as tc:
        with (
            tc.tile_pool(name="wpool", bufs=1) as wpool,
            tc.tile_pool(name="xpool", bufs=3) as xpool,
            tc.tile_pool(name="opool", bufs=4) as opool,
            tc.tile_pool(name="bpool", bufs=1) as bpool,
            tc.tile_pool(name="psum", bufs=8, space="PSUM") as psum_pool,
        ):
            # Whole weight matrix resident in SBUF: [128, KT*D] fp32 = 64KB/partition.
            # One DMA per k-tile so each matmul waits on at most one DMA queue.
            # Interleave slab-0 x DMAs with the weight DMAs so the PE can start
            # ~2us in instead of waiting for all 8.4MB of weights first.
            w_sb = wpool.tile([P, KT * D], w_dt)
            x_sb0 = xpool.tile([P, KT * NT], x_dt)
            for t in range(KT):
                nc.sync.dma_start(
                    out=x_sb0[:, t * NT : (t + 1) * NT], in_=xT_v[:, t, 0:NT]
                )
                nc.sync.dma_start(out=w_sb[:, t * D : (t + 1) * D], in_=wT_v[:, t, :])
            bias_sb = bpool.tile([P, JT], FP32)
            nc.sync.dma_start(out=bias_sb[:], in_=biasT[:, :])

            def store(j, bi, ps):
                o_sb = opool.tile([P, NT], FP32)
                nc.vector.tensor_scalar_add(o_sb[:], ps[:], bias_sb[:, j : j + 1])
                nc.sync.dma_start(out=yT_v[j, :, bi * NT : (bi + 1) * NT], in_=o_sb[:])

            # All slabs j-outer; the scheduler interleaves ready matmuls
            # across the 8 open psum groups as k-slices arrive.
            # Loads for slab bi+1 are issued BEFORE slab bi's stores: the sync
            # sequencer is in-order, so a store dma_start waiting on its DVE
            # bias-add would otherwise head-of-line-block the next slab's x
            # stream (seen as mid-kernel LDWEIGHTS stalls on DMA sems).
            x_tiles = [x_sb0] + [None] * (BT - 1)

            def load_slab(bi):
                xt = xpool.tile([P, KT * NT], x_dt, tag="x_sb0", name="x_sb")
                for t in range(KT):
                    nc.sync.dma_start(
                        out=xt[:, t * NT : (t + 1) * NT],
                        in_=xT_v[:, t, bi * NT : (bi + 1) * NT],
                    )
                x_tiles[bi] = xt

            for bi in range(BT):
                if bi + 1 < BT:
                    load_slab(bi + 1)
                x_sb = x_tiles[bi]
                for j in range(JT):
                    ps = psum_pool.tile([P, NT], FP32, tag="ps")
                    for t in range(KT):
                        nc.tensor.matmul(
                            ps[:],
                            w_sb[:, t * D + j * P : t * D + (j + 1) * P],
                            x_sb[:, t * NT : (t + 1) * NT],
                            start=(t == 0),
                            stop=(t == KT - 1),
                        )
                    store(j, bi, ps)
    nc.finalize()
    return nc


def kernel(feat_rgb, feat_depth, W_k1, b_k1, W_k2, b_k2, W_f, b_f):
    global LAST_RESULT
    feat_rgb = np.asarray(feat_rgb, dtype=np.float32)
    feat_depth = np.asarray(feat_depth, dtype=np.float32)
    W_f = np.asarray(W_f, dtype=np.float32)
    b_f = np.asarray(b_f, dtype=np.float32)

    _, _, w_np, x_np = _dtypes()
    WfT = np.ascontiguousarray(W_f.T).astype(w_np)  # [2048, 1024]
    biasT = np.ascontiguousarray(b_f.reshape(JT, P).T)  # [128, 8]

    in_maps = []
    for i in range(NCORES):
        lo, hi = i * BLOC, (i + 1) * BLOC
        x_cat_T = np.empty((K, BLOC), dtype=x_np)
        x_cat_T[:D] = feat_depth[lo:hi].T
        x_cat_T[D:] = feat_rgb[lo:hi].T
        in_maps.append({"xT": x_cat_T, "wT": WfT, "biasT": biasT})

    nc = _build_nc()
    res = run_bass_kernel_spmd(
        nc, in_maps, list(range(NCORES)), trace=TRACE, tmpdir=TRACE_DIR
    )
    LAST_RESULT = res

    out = np.empty((B, D), dtype=np.float32)
    for i in range(NCORES):
        out[i * BLOC : (i + 1) * BLOC] = res.results[i]["yT"].T
    return out



# revision 3
# speedup vs baseline: 1.0902x; 1.0902x over previous
"""Mixed-precision Bass kernel for nn_CoAttentionFusionBlock.

Math: out = concat([feat_depth, feat_rgb]) @ W_f.T + b_f (the reference's
softmax is over a singleton dim -> weights exactly 1, score path dead).

The PE runs 1 moving col/cycle for bf16 but 2 contraction slots/cycle in fp8
DoubleRow mode (measured: both issue N=512 matmuls at 216 ns). Pure fp8 e4m3
fails the 2e-2 gate (~3.5e-2), so the K=2048 contraction is split:

  k 0..1535  (depth + rgb[:,:512])  bf16, 12 k-tiles, exact-ish
  k 1536..2047 (rgb[:,512:])        fp8 e4m3 DoubleRow, 2 k-tiles of 256

Quantization error scales as 3.5e-2*sqrt(512/2048) ~ 1.8e-2 < 2e-2, while the
matmul stream shrinks from 16 to 14 MMs per output tile (-12.5%).

Weights for the fp8 section are scaled x64 on host (else half of W_f lands in
e4m3 subnormals); the epilogue rescales: out = ps_bf + ps_fp/64 + bias via
ACT (Identity, scale+bias from PSUM) + DVE (tensor_tensor add).

A chain of ~120 tiny matmuls on memset data warms the PE's HAM clock gate
during the ~10us DMA-gated head so real matmuls start at 2.4 GHz.
"""

import numpy as np
import ml_dtypes

import concourse.bacc as bacc
import concourse.mybir as mybir
import concourse.tile as tile
from concourse.bass_utils import run_bass_kernel_spmd

B = 32768
D = 1024
NCORES = 8
BLOC = B // NCORES  # 4096
K = 2 * D
P = 128
NT = 512
JT = D // P  # 8
BT = BLOC // NT  # 8

KBT = 12  # bf16 k-tiles (128 k each)
KFT = 2   # fp8 DoubleRow k-tiles (256 k each)
KB = KBT * P          # 1536
KF = K - KB           # 512
WSCALE = 64.0
N_WARMUP = 32

FP32 = mybir.dt.float32
BF16 = mybir.dt.bfloat16
E4 = mybir.dt.float8e4
E4NP = np.dtype(ml_dtypes.float8_e4m3)
BF16NP = np.dtype(ml_dtypes.bfloat16)

TRACE = False
TRACE_DIR = None
LAST_RESULT = None
DT_IN = "mixed-bf16-fp8dr"  # informational; test.py prints this


def _build_nc():
    nc = bacc.Bacc(None)
    xbT = nc.declare_dram_parameter("xbT", [KB, BLOC], BF16, isOutput=False)
    xfT = nc.declare_dram_parameter("xfT", [KF, BLOC], E4, isOutput=False)
    wbT = nc.declare_dram_parameter("wbT", [KB, D], BF16, isOutput=False)
    wfT = nc.declare_dram_parameter("wfT", [KF, D], E4, isOutput=False)
    biasT = nc.declare_dram_parameter("biasT", [P, JT], FP32, isOutput=False)
    yT = nc.declare_dram_parameter("yT", [D, BLOC], FP32, isOutput=True)

    xb_v = xbT.rearrange("(t p) b -> p t b", p=P)            # [128,KBT,BLOC]
    xf_v = xfT.rearrange("(t ko p) b -> p ko t b", ko=2, p=P)  # [128,2,KFT,BLOC]
    wb_v = wbT.rearrange("(t p) j -> p t j", p=P)            # [128,KBT,D]
    wf_v = wfT.rearrange("(t ko p) j -> p ko t j", ko=2, p=P)  # [128,2,KFT,D]
    yT_v = yT.rearrange("(j p) b -> j p b", p=P)             # [JT,128,BLOC]

    DR = mybir.MatmulPerfMode.DoubleRow

    with tile.TileContext(nc) as tc:
        with (
            tc.tile_pool(name="wu", bufs=1) as wupool,
            tc.tile_pool(name="wpool", bufs=1) as wpool,
            tc.tile_pool(name="xpool", bufs=4) as xpool,
            tc.tile_pool(name="tpool", bufs=4) as tpool,
            tc.tile_pool(name="opool", bufs=4) as opool,
            tc.tile_pool(name="bpool", bufs=1) as bpool,
            tc.tile_pool(name="psum", bufs=4, space="PSUM") as psum_pool,
        ):
            # --- PE warm-up: keep the HAM clock gate busy during the DMA
            # head. WAW chain on one psum tile serializes the stream.
            wu_sb = wupool.tile([P, 2 * P], BF16)
            nc.vector.memset(wu_sb[:], 0)
            wu_ps = psum_pool.tile([P, NT], FP32, tag="ps_a")
            for _ in range(N_WARMUP):
                nc.tensor.matmul(
                    wu_ps[:, :P], wu_sb[:, :P], wu_sb[:, P : 2 * P],
                    start=True, stop=True,
                )

            # --- weights resident in SBUF; interleave with slab-0 x loads so
            # the PE can start as soon as the first (w,x) k-tile pair lands.
            bias_sb = bpool.tile([P, JT], FP32)
            nc.sync.dma_start(out=bias_sb[:], in_=biasT[:, :])
            w_b = wpool.tile([P, KBT * D], BF16)
            w_f = wpool.tile([P, 2, KFT * D], E4)
            xb0 = xpool.tile([P, KBT * NT], BF16)
            xf0 = xpool.tile([P, 2, KFT * NT], E4)
            for t in range(KBT):
                nc.sync.dma_start(out=xb0[:, t * NT : (t + 1) * NT],
                                  in_=xb_v[:, t, 0:NT])
                nc.sync.dma_start(out=w_b[:, t * D : (t + 1) * D],
                                  in_=wb_v[:, t, :])
            for t in range(KFT):
                nc.sync.dma_start(out=xf0[:, :, t * NT : (t + 1) * NT],
                                  in_=xf_v[:, :, t, 0:NT])
                nc.sync.dma_start(out=w_f[:, :, t * D : (t + 1) * D],
                                  in_=wf_v[:, :, t, :])

            def epilogue(j, bi, ps_a, ps_b):
                # t1 = ps_b/WSCALE + bias_j on the ACT engine, o = ps_a + t1
                # on the DVE; store.
                t1 = tpool.tile([P, NT], FP32)
                nc.scalar.activation(
                    t1[:], ps_b[:], mybir.ActivationFunctionType.Identity,
                    bias=bias_sb[:, j : j + 1], scale=1.0 / WSCALE,
                )
                o_sb = opool.tile([P, NT], FP32)
                nc.vector.tensor_tensor(
                    o_sb[:], ps_a[:], t1[:], mybir.AluOpType.add
                )
                nc.sync.dma_start(out=yT_v[j, :, bi * NT : (bi + 1) * NT],
                                  in_=o_sb[:])

            xb_tiles = [xb0] + [None] * (BT - 1)
            xf_tiles = [xf0] + [None] * (BT - 1)

            def load_slab(bi):
                xb = xpool.tile([P, KBT * NT], BF16, tag="xb0", name="xb_sb")
                xf = xpool.tile([P, 2, KFT * NT], E4, tag="xf0", name="xf_sb")
                lo = bi * NT
                for t in range(KBT):
                    nc.sync.dma_start(out=xb[:, t * NT : (t + 1) * NT],
                                      in_=xb_v[:, t, lo : lo + NT])
                for t in range(KFT):
                    nc.sync.dma_start(out=xf[:, :, t * NT : (t + 1) * NT],
                                      in_=xf_v[:, :, t, lo : lo + NT])
                xb_tiles[bi] = xb
                xf_tiles[bi] = xf

            for bi in range(BT):
                if bi + 1 < BT:
                    load_slab(bi + 1)
                xb, xf = xb_tiles[bi], xf_tiles[bi]
                for j in range(JT):
                    ps_a = psum_pool.tile([P, NT], FP32, tag="ps_a")
                    for t in range(KBT):
                        nc.tensor.matmul(
                            ps_a[:],
                            w_b[:, t * D + j * P : t * D + (j + 1) * P],
                            xb[:, t * NT : (t + 1) * NT],
                            start=(t == 0), stop=(t == KBT - 1),
                        )
                    ps_b = psum_pool.tile([P, NT], FP32, tag="ps_b")
                    for t in range(KFT):
                        nc.tensor.matmul(
                            ps_b[:],
                            w_f[:, :, t * D + j * P : t * D + (j + 1) * P],
                            xf[:, :, t * NT : (t + 1) * NT],
                            start=(t == 0), stop=(t == KFT - 1),
                            perf_mode=DR,
                        )
                    epilogue(j, bi, ps_a, ps_b)
    nc.finalize()
    return nc


def kernel(feat_rgb, feat_depth, W_k1, b_k1, W_k2, b_k2, W_f, b_f):
    global LAST_RESULT
    feat_rgb = np.asarray(feat_rgb, dtype=np.float32)
    feat_depth = np.asarray(feat_depth, dtype=np.float32)
    W_f = np.asarray(W_f, dtype=np.float32)
    b_f = np.asarray(b_f, dtype=np.float32)

    WfT = np.ascontiguousarray(W_f.T)  # [2048, 1024]
    wbT = WfT[:KB].astype(BF16NP)
    wfT = np.ascontiguousarray(WfT[KB:] * WSCALE).astype(E4NP)
    biasT = np.ascontiguousarray(b_f.reshape(JT, P).T)

    in_maps = []
    for i in range(NCORES):
        lo, hi = i * BLOC, (i + 1) * BLOC
        xbT = np.empty((KB, BLOC), dtype=BF16NP)
        xbT[:D] = feat_depth[lo:hi].T.astype(BF16NP)
        xbT[D:] = feat_rgb[lo:hi, : KB - D].T.astype(BF16NP)
        xfT = np.ascontiguousarray(feat_rgb[lo:hi, KB - D :].T).astype(E4NP)
        in_maps.append(
            {"xbT": xbT, "xfT": xfT, "wbT": wbT, "wfT": wfT, "biasT": biasT}
        )

    nc = _build_nc()
    res = run_bass_kernel_spmd(
        nc, in_maps, list(range(NCORES)), trace=TRACE, tmpdir=TRACE_DIR
    )
    LAST_RESULT = res

    out = np.empty((B, D), dtype=np.float32)
    for i in range(NCORES):
        out[i * BLOC : (i + 1) * BLOC] = res.results[i]["yT"].T
    return out
